# revision 15
# baseline (speedup 1.0000x reference)
"""CLAHE Bass kernel for trn2 — one NeuronCore processes 4 images (12 planes)."""
import sys
sys.path.insert(0, '/opt/trn_rl_repo')
import re
from collections import defaultdict
from contextlib import ExitStack

import numpy as np
import bass_rust
import concourse.bass as bass
import concourse.tile as tile
import concourse.mybir as mybir

f32 = mybir.dt.float32
bf16 = mybir.dt.bfloat16
i16 = mybir.dt.int16
u16 = mybir.dt.uint16
u32 = mybir.dt.uint32
u8 = mybir.dt.uint8
Alu = mybir.AluOpType
ts = bass.ts

H = W = 512
NPLANES = 12
PIX = H * W
MAXVAL = 640.0
SCL = float(np.float32(255.0) / np.float32(4096.0))

# ---------------- static geometry ----------------
def _bands():
    out = [(0, 32, 0, 0)]
    for k in range(1, 8):
        out.append((32 + 64 * (k - 1), 64, k - 1, k))
    out.append((480, 32, 7, 7))
    return out

BANDS = _bands()

# block order = (by, bx) row-major, 9x9
BLOCKS = []
for by, (y0, hgt, ty0, ty1) in enumerate(BANDS):
    for bx, (x0, wid, tx0, tx1) in enumerate(BANDS):
        BLOCKS.append(dict(by=by, bx=bx, y0=y0, x0=x0, h=hgt, w=wid,
                           ty0=ty0, ty1=ty1, tx0=tx0, tx1=tx1, n=hgt * wid))

def _slot_map():
    fulls = [i for i, b in enumerate(BLOCKS) if b['n'] == 4096]
    rowh = [i for i, b in enumerate(BLOCKS) if b['n'] == 2048 and b['h'] == 32]
    colh = [i for i, b in enumerate(BLOCKS) if b['n'] == 2048 and b['h'] == 64]
    quarts = [i for i, b in enumerate(BLOCKS) if b['n'] == 1024]
    assert len(rowh) == 14 and len(colh) == 14
    half_w64 = [[(i, 0)] for i in fulls]                   # 49 rows
    for k in range(7):                                     # 7 rows (32x64 pairs)
        half_w64.append([(rowh[k], 0), (rowh[k + 7], 2048)])
    colh0 = [i for i in colh if BLOCKS[i]['bx'] == 0]
    colh8 = [i for i in colh if BLOCKS[i]['bx'] == 8]
    half_w32 = []
    for k in range(7):                                     # 7 rows (64x32 pairs)
        half_w32.append([(colh0[k], 0), (colh8[k], 2048)])
    half_w32.append([(q, 1024 * i) for i, q in enumerate(quarts)])
    return half_w64, half_w32   # 56 + 8 rows per half

HALF_W64, HALF_W32 = _slot_map()
# global rows: 0-111 w64 (half0 then half1), 112-127 w32 (half0 then half1)
SLOT_OF = {}   # (half, block_idx) -> (row, off)
for half in range(2):
    for r, members in enumerate(HALF_W64):
        for (bi, off) in members:
            SLOT_OF[(half, bi)] = (56 * half + r, off)
    for r, members in enumerate(HALF_W32):
        for (bi, off) in members:
            SLOT_OF[(half, bi)] = (112 + 8 * half + r, off)

def _wband(pos0, size):
    pos = np.arange(pos0, pos0 + size)
    r = pos - 32
    inside = (pos >= 32) & (pos < 480)
    k = np.clip(r // 64, 0, 6)
    rr = r - k * 64
    return np.where(inside, (64 - 1 - rr).astype(np.float32) / np.float32(63.0),
                    np.float32(1.0)).astype(np.float32)

def make_weight_consts():
    wx = np.zeros((128, 4096), np.float32)
    wy = np.zeros((128, 4096), np.float32)
    for half in range(2):
        for bi, b in enumerate(BLOCKS):
            s, off = SLOT_OF[(half, bi)]
            wyb = _wband(b['y0'], b['h'])
            wxb = _wband(b['x0'], b['w'])
            n = b['n']
            i = np.arange(n)
            r = i % 16
            rest = i // 16
            x = rest % b['w']
            yq = rest // b['w']
            wx[s, off:off + n] = wxb[x]
            wy[s, off:off + n] = wyb[16 * yq + r]
    return wx.copy(), wy.copy()

# ---------------- walrus sync fixups ----------------
def _mk_nop(name, engine, waits=(), updates=()):
    nop = mybir.InstNoOp(name=name, ins=[], outs=[])
    nop.engine = engine
    nop.sync_info = bass_rust.SyncInfo(on_wait=list(waits), on_update=list(updates))
    return nop

def fixup_sync(nc, max_waits=1):
    ctr = [0]
    fwait = defaultdict(int)
    lwait = defaultdict(int)

    def nop(engine, waits=(), updates=()):
        ctr[0] += 1
        return _mk_nop(f"SFIX-{ctr[0]}", engine, waits, updates)

    for fn in nc.m.functions:
        for bb in fn.blocks:
            new_list, changed = [], False
            for ins in bb.instructions:
                si = ins.sync_info
                if si is None or (not si.on_wait and not si.on_update):
                    new_list.append(ins)
                    continue
                waits = list(si.on_wait)
                updates = list(si.on_update)
                nw = []
                for w in waits:
                    nm = w.ant_name or ''
                    if w.wait_mode == 'sem-eq-imm' and 'barrier_' in nm:
                        changed = True
                        continue
                    if 'barrier_' in nm and w.wait_mode == 'sem-ge-imm':
                        kind = 'release' if nm.endswith('_release') else 'gather'
                        n1 = len(re.match(rf'barrier_(.*)_{kind}', nm).group(1).split('_')) - 1
                        d = fwait if kind == 'release' else lwait
                        k = d[(nm, ins.engine)]; d[(nm, ins.engine)] += 1
                        nw.append(bass_rust.SyncWait(sync_type='semaphore', id=w.id,
                            ant_name=nm, wait_mode='sem-ge-imm', wait_value=(k + 1) * n1,
                            wait_reg=None))
                        changed = True
                        continue
                    assert w.wait_mode in ('sem-ge-imm', 'sem-ge-reg'), \
                        f"unhandled wait mode {w.wait_mode} on {ins.opcode} {ins.name} ({nm})"
                    nw.append(w)
                nu = []
                for uu in updates:
                    nm = uu.ant_name or ''
                    if 'barrier_' in nm and uu.update_mode in ('sem-dec', 'sem-sub-imm'):
                        changed = True
                        continue
                    nu.append(uu)
                waits, updates = nw, nu
                if ins.opcode == 'Drain':
                    changed = True
                    rest = waits
                    while rest:
                        chunk, rest = rest[:max_waits], rest[max_waits:]
                        new_list.append(nop(ins.engine, waits=chunk))
                    ins.sync_info = bass_rust.SyncInfo(on_wait=[], on_update=[])
                    new_list.append(ins)
                    if updates:
                        new_list.append(nop(ins.engine, updates=updates))
                    continue
                if len(waits) > max_waits:
                    changed = True
                    keep, rest = waits[:max_waits], waits[max_waits:]
                    while rest:
                        chunk, rest = rest[:max_waits], rest[max_waits:]
                        new_list.append(nop(ins.engine, waits=chunk))
                    waits = keep
                ins.sync_info = bass_rust.SyncInfo(on_wait=waits, on_update=updates)
                new_list.append(ins)
            if changed:
                bb.instructions.clear()
                bb.instructions.extend(new_list)
    return ctr[0]

def _ap(base, ap_list, off=0):
    return bass.AP(tensor=base.tensor, offset=base.offset + off, ap=ap_list)

# ---------------- kernel ----------------
def build(debug=False):
    nc = bass.Bass("TRN2")
    img_in = nc.declare_dram_parameter("img", [NPLANES, H, W], f32, isOutput=False)
    wx_in = nc.declare_dram_parameter("wxc", [128, 4096], bf16, isOutput=False)
    wy_in = nc.declare_dram_parameter("wyc", [128, 4096], bf16, isOutput=False)
    out_ext = nc.declare_dram_parameter("out", [NPLANES, H, W], f32, isOutput=True)
    dbg = {}
    if debug:
        dbg['hist'] = nc.declare_dram_parameter("dbg_hist", [NPLANES, 64, 256], f32, isOutput=True)
        dbg['lut'] = nc.declare_dram_parameter("dbg_lut", [NPLANES, 64, 256], f32, isOutput=True)

    with tile.TileContext(nc) as tc, ExitStack() as ctx:
        dram_pool = ctx.enter_context(tc.tile_pool(name="drsc", bufs=1, space="DRAM"))
        hist_dram_ts = [dram_pool.tile([1, 64 * 16384], f32, tag=f"histsc{p}", name=f"histsc{p}")
                        for p in range(NPLANES)]
        lutp_ts = [dram_pool.tile([1, 2 * 25600], u8, tag=f"lutp{pp}", name=f"lutp{pp}")
                   for pp in range(NPLANES // 2)]
        hist_dram = lambda p: hist_dram_ts[p][:]
        lutp_dram = lambda p: lutp_ts[p // 2][:]
        cpool = ctx.enter_context(tc.tile_pool(name="const", bufs=1))
        iota_j = cpool.tile([128, 512], i16)      # (c:4, j:16, i:8) -> j
        nc.gpsimd.iota(iota_j[:], pattern=[[0, 4], [1, 16], [0, 8]], base=0, channel_multiplier=0)
        iota_v = cpool.tile([64, 256], i16)
        nc.gpsimd.iota(iota_v[:], pattern=[[1, 256]], base=0, channel_multiplier=0)
        zero256 = cpool.tile([64, 256], f32)
        nc.vector.memset(zero256[:], 0.0)
        wxc = cpool.tile([128, 4096], bf16)
        nc.sync.dma_start(wxc[:], wx_in[:])
        wyc = cpool.tile([128, 4096], bf16)
        nc.sync.dma_start(wyc[:], wy_in[:])

        imgH_pool = ctx.enter_context(tc.tile_pool(name="imgH", bufs=1))
        prep_pool = ctx.enter_context(tc.tile_pool(name="prep", bufs=1))
        gen_pool = ctx.enter_context(tc.tile_pool(name="gen", bufs=1))
        psum_pool = ctx.enter_context(tc.tile_pool(name="ps", bufs=3, space="PSUM"))
        lut_pool = ctx.enter_context(tc.tile_pool(name="lut", bufs=1))
        lut8_pool = ctx.enter_context(tc.tile_pool(name="lut8", bufs=2))
        tab_pool = ctx.enter_context(tc.tile_pool(name="tab", bufs=1))
        gat_pool = ctx.enter_context(tc.tile_pool(name="gat", bufs=1))
        dense_pool = ctx.enter_context(tc.tile_pool(name="dense", bufs=2))
        blend_pool = ctx.enter_context(tc.tile_pool(name="blend", bufs=1))

        lut8_all = []

        # ======== per-plane: histogram -> LUT(u8) ========
        for p in range(NPLANES):
            imgH = imgH_pool.tile([128, 2048], f32, tag="imgH")
            for hh in range(2):
                for band in range(8):
                    src = _ap(img_in[:], [[W, 64], [64, 8], [1, 32]],
                              off=p * PIX + band * 64 * W + 32 * hh)
                    nc.sync.dma_start(imgH[64 * hh:64 * hh + 64, ts(band, 256)], src)
            v256 = prep_pool.tile([128, 2048], i16, tag="v256")
            nc.scalar.activation(v256[:], imgH[:], mybir.ActivationFunctionType.Copy,
                                 bias=-0.5, scale=256.0)
            hi16 = prep_pool.tile([128, 2048], i16, tag="hi16")
            nc.scalar.activation(hi16[:], imgH[:], mybir.ActivationFunctionType.Copy,
                                 bias=-0.5, scale=16.0)
            lo16 = prep_pool.tile([128, 2048], i16, tag="lo16")
            nc.vector.scalar_tensor_tensor(out=lo16[:], in0=hi16[:], scalar=-16.0,
                                           in1=v256[:], op0=Alu.mult, op1=Alu.add)

            for band in range(8):
                psum = psum_pool.tile([128, 1024], f32, tag="psum")
                for tcol in range(8):
                    off = (band * 8 + tcol) * 32
                    himat = gen_pool.tile([128, 512], bf16, tag="himat")
                    lomat = gen_pool.tile([128, 512], bf16, tag="lomat")
                    hi_b = _ap(hi16[:], [hi16[:].ap[0], [8, 4], [0, 16], [1, 8]], off=off)
                    lo_b = _ap(lo16[:], [lo16[:].ap[0], [8, 4], [0, 16], [1, 8]], off=off)
                    d3 = [[128, 4], [8, 16], [1, 8]]
                    hm3 = _ap(himat[:], [himat[:].ap[0]] + d3)
                    lm3 = _ap(lomat[:], [lomat[:].ap[0]] + d3)
                    io3 = _ap(iota_j[:], [iota_j[:].ap[0]] + d3)
                    nc.vector.tensor_tensor(hm3, io3, hi_b, Alu.is_equal)
                    nc.vector.tensor_tensor(lm3, io3, lo_b, Alu.is_equal)
                    for c in range(4):
                        nc.tensor.matmul(psum[:, ts(tcol, 128)],
                                         lomat[:, ts(c, 128)], himat[:, ts(c, 128)],
                                         start=(c == 0), stop=(c == 3))
                pb = lut_pool.tile([128, 1024], f32, tag="pbounce")
                (nc.vector.tensor_copy if band % 2 else nc.scalar.copy)(pb[:], psum[:])
                dst = _ap(hist_dram(p), [[128, 128], [16384, 8], [1, 128]],
                          off=band * 8 * 16384)
                nc.sync.dma_start(dst, pb[:])

            # extraction: hist[v=16hi+lo] = sum_a hist4[(8lo+a)*128 + 8hi+a]
            hist = lut_pool.tile([64, 256], f32, tag="hist")
            for chunk in range(4):   # lo in [4*chunk, 4*chunk+4)
                h4 = lut_pool.tile([64, 4096], f32, tag="h4")
                nc.sync.dma_start(h4[:], _ap(hist_dram(p), [[16384, 64], [1, 4096]],
                                             off=chunk * 4096))
                for a in range(8):
                    sl = _ap(h4[:], [h4[:].ap[0], [8, 16], [1024, 4]], off=129 * a)
                    dsthist = _ap(hist[:], [hist[:].ap[0], [16, 16], [1, 4]], off=4 * chunk)
                    if a == 0:
                        nc.vector.tensor_copy(dsthist, sl)
                    else:
                        nc.vector.tensor_tensor(dsthist, _ap(hist[:], [hist[:].ap[0], [16, 16], [1, 4]], off=4 * chunk), sl, Alu.add)

            # clip + redistribute + cdf -> LUT
            nc.vector.tensor_scalar(hist[:], hist[:], MAXVAL, None, Alu.min)
            hsum = lut_pool.tile([64, 1], f32, tag="hsum")
            nc.vector.tensor_reduce(hsum[:], hist[:], mybir.AxisListType.X, Alu.add)
            excess = lut_pool.tile([64, 1], f32, tag="excess")
            nc.vector.tensor_scalar(excess[:], hsum[:], -1.0, 4096.0, Alu.mult, Alu.add)
            redq = lut_pool.tile([64, 1], i16, tag="redq")
            nc.vector.tensor_scalar(redq[:], excess[:], 1.0 / 256.0, -0.5 + 2.0**-13,
                                    Alu.mult, Alu.add)
            redist = lut_pool.tile([64, 1], f32, tag="redist")
            nc.vector.tensor_copy(redist[:], redq[:])
            residual = lut_pool.tile([64, 1], f32, tag="residual")
            nc.vector.scalar_tensor_tensor(out=residual[:], in0=redist[:], scalar=-256.0,
                                           in1=excess[:], op0=Alu.mult, op1=Alu.add)
            nc.vector.tensor_scalar(hist[:], hist[:], redist[:], None, Alu.add)
            bump = lut_pool.tile([64, 256], f32, tag="bump")
            nc.vector.tensor_scalar(bump[:], iota_v[:], residual[:], None, Alu.is_lt)
            nc.vector.tensor_tensor(hist[:], hist[:], bump[:], Alu.add)
            if debug:
                nc.sync.dma_start(_ap(dbg['hist'][:], [[256, 64], [1, 256]], off=p * 64 * 256), hist[:])
            cdf = lut_pool.tile([64, 256], f32, tag="cdf")
            nc.vector.tensor_tensor_scan(cdf[:], hist[:], zero256[:], 0.0, Alu.add, Alu.add)
            lutf = lut_pool.tile([64, 256], f32, tag="lutf")
            nc.vector.tensor_scalar(lutf[:], cdf[:], SCL, 255.0, Alu.mult, Alu.min)
            lut8 = lut8_pool.tile([64, 256], u8, tag="lut8")
            nc.vector.tensor_scalar(lut8[:], lutf[:], -(0.5 - 2.0**-13), None, Alu.add)
            if debug:
                lutdf = lut_pool.tile([64, 256], f32, tag="lutdf")
                nc.vector.tensor_copy(lutdf[:], lut8[:])
                nc.sync.dma_start(_ap(dbg['lut'][:], [[256, 64], [1, 256]], off=p * 64 * 256), lutdf[:])
            lut8_all.append(lut8)

            # write clamp-padded LUT to DRAM: lutp[p, i, j] = LUT[clip(i-1,0,7), clip(j-1,0,7)]
            lp = lutp_dram(p)
            pbase = (p % 2) * 25600
            # interior rows i=1..8, j=1..8  (src = all 64 tiles)
            nc.sync.dma_start(_ap(lp, [[2560, 8], [256, 8], [1, 256]], off=pbase + 2560 + 256),
                              lut8[:])
            for (i_, jsl, srcap) in [
                (0, (1, 8), lut8[0:8, :]),
                (9, (1, 8), lut8[56:64, :]),
            ]:
                nc.sync.dma_start(_ap(lp, [[256, 8], [1, 256]], off=pbase + i_ * 2560 + 256), srcap)
            # col pads via DRAM->DRAM: col 0 <- col 1, col 9 <- col 8 (all 10 rows)
            nc.sync.dma_start(_ap(lp, [[2560, 10], [1, 256]], off=pbase),
                              _ap(lp, [[2560, 10], [1, 256]], off=pbase + 256))
            nc.sync.dma_start(_ap(lp, [[2560, 10], [1, 256]], off=pbase + 9 * 256),
                              _ap(lp, [[2560, 10], [1, 256]], off=pbase + 8 * 256))

        # ======== gather + blend per plane pair ========
        # Exact-packed entries: every gather call is 8 groups x 4096 indices.
        # Each group's table is 4 stacked 1024B chunks (4x256 u32 entries);
        # the index for stream chunk c is v + 256*c (bias folded into the
        # v255 activation), so pairs stack [A,A,B,B], quarters [q0..q3],
        # fulls [T,T,T,T].
        fulls = [i for i, b in enumerate(BLOCKS) if b['n'] == 4096]
        rowh = [i for i, b in enumerate(BLOCKS) if b['n'] == 2048 and b['h'] == 32]
        colh0 = [i for i, b in enumerate(BLOCKS) if b['n'] == 2048 and b['h'] == 64 and b['bx'] == 0]
        colh8 = [i for i, b in enumerate(BLOCKS) if b['n'] == 2048 and b['h'] == 64 and b['bx'] == 8]
        quars = [i for i, b in enumerate(BLOCKS) if b['n'] == 1024]
        entries = []
        for e in range(6):
            entries.append([[fulls[8 * e + j]] * 4 for j in range(8)])
        entries.append([[fulls[48]] * 4]
                       + [[rowh[k], rowh[k], rowh[k + 7], rowh[k + 7]] for k in range(7)])
        entries.append([[colh0[k], colh0[k], colh8[k], colh8[k]] for k in range(7)]
                       + [quars])

        def dense_row(half, e, j):
            if e < 6:
                return 56 * half + 8 * e + j
            if e == 6:
                return 56 * half + 48 + j
            return 112 + 8 * half + j

        for pp in range(NPLANES // 2):
            dense = dense_pool.tile([128, 4096], u32, tag="dense")
            for half in range(2):
                p = 2 * pp + half
                pbase = (p % 2) * 25600
                # corner staging Xc[bi, ci*256+v] via 4 affine reads of lutp
                Xc = tab_pool.tile([81, 1024], u8, tag="Xc")
                for ci, (dy, dx) in enumerate([(0, 0), (0, 1), (1, 0), (1, 1)]):
                    src = _ap(lutp_dram(p), [[2560, 9], [256, 9], [1, 256]],
                              off=pbase + dy * 2560 + dx * 256)
                    nc.sync.dma_start(Xc[0:81, ci * 256:(ci + 1) * 256], src)
                Tq = tab_pool.tile([81, 1024], u8, tag="Tq", bufs=2)
                for ci in range(4):
                    nc.vector.tensor_copy(_ap(Tq[:], [Tq[:].ap[0], [4, 256]], off=ci),
                                          Xc[:, ts(ci, 256)])
                for e, egroups in enumerate(entries):
                    # chunk-call k reads table slice smap[k]; pure-full entries
                    # share one 1KB slice (cheaper per-call table staging)
                    smap = [0, 0, 0, 0] if e < 6 else [0, 1, 2, 3]
                    Tg = tab_pool.tile([128, 4096], u8, tag="Tg", bufs=2)
                    imgG = gat_pool.tile([128, 256], f32, tag="imgG", bufs=2)
                    for j, stack in enumerate(egroups):
                        # stage each distinct table at slice smap[c]
                        staged = set()
                        c = 0
                        while c < 4:
                            r = c
                            while r + 1 < 4 and stack[r + 1] == stack[c]:
                                r += 1
                            rep = r - c + 1
                            bi = stack[c]
                            s0, s1 = smap[c], smap[r]
                            if (bi, s0) not in staged:
                                staged.add((bi, s0))
                                nrep = s1 - s0 + 1
                                srcT = _ap(Tq[bi:bi + 1, :],
                                           [Tq[bi:bi + 1, :].ap[0], [0, nrep], [1, 1024]])
                                nc.scalar.dma_start(
                                    Tg[16 * j:16 * j + 1, 1024 * s0:1024 * (s0 + nrep)],
                                    srcT)
                            b = BLOCKS[bi]
                            nyq = b['h'] // 16
                            src = _ap(img_in[:], [[W, 16], [16 * W, nyq], [1, b['w']]],
                                      off=p * PIX + b['y0'] * W + b['x0'])
                            nc.scalar.dma_start(
                                imgG[16 * j:16 * j + 16, 64 * c:64 * c + 64 * rep], src)
                            c = r + 1
                    v255 = gat_pool.tile([128, 256], u16, tag="v255", bufs=3)
                    nc.scalar.activation(v255[:], imgG[:],
                                         mybir.ActivationFunctionType.Copy,
                                         bias=-0.5, scale=255.0)
                    gout = gat_pool.tile([128, 4096], u32, tag="gout", bufs=2)
                    for k in range(4):
                        dslice = Tg[:, 1024 * smap[k]:1024 * smap[k] + 1024]
                        nc.gpsimd.indirect_copy(gout[:, k * 1024:(k + 1) * 1024],
                                                dslice.bitcast(u32),
                                                v255[:, 64 * k:64 * (k + 1)], True)
                    for j in range(8):
                        s = dense_row(half, e, j)
                        nc.sync.dma_start(dense[s:s + 1, 0:4096],
                                          gout[16 * j:16 * j + 1, 0:4096])

            # ---- unpack + blend ----
            dq = dense[:].bitcast(u8)
            gA = blend_pool.tile([128, 4096], bf16, tag="gA")
            gB = blend_pool.tile([128, 4096], bf16, tag="gB")
            ttop = blend_pool.tile([128, 4096], f32, tag="ttop")
            nc.vector.tensor_copy(gA[:], _ap(dq, [dq.ap[0], [4, 4096]], off=0))
            nc.scalar.copy(gB[:], _ap(dq, [dq.ap[0], [4, 4096]], off=1))
            nc.vector.tensor_tensor(gA[:], gA[:], gB[:], Alu.subtract)   # g00-g01 (exact in bf16)
            nc.vector.tensor_tensor(ttop[:], gA[:], wxc[:], Alu.mult)
            nc.vector.tensor_tensor(ttop[:], ttop[:], gB[:], Alu.add)
            gC = blend_pool.tile([128, 4096], bf16, tag="gA")
            gD = blend_pool.tile([128, 4096], bf16, tag="gB")
            tbot = blend_pool.tile([128, 4096], f32, tag="tbot")
            nc.vector.tensor_copy(gC[:], _ap(dq, [dq.ap[0], [4, 4096]], off=2))
            nc.scalar.copy(gD[:], _ap(dq, [dq.ap[0], [4, 4096]], off=3))
            nc.vector.tensor_tensor(gC[:], gC[:], gD[:], Alu.subtract)
            nc.vector.tensor_tensor(tbot[:], gC[:], wxc[:], Alu.mult)
            nc.vector.tensor_tensor(tbot[:], tbot[:], gD[:], Alu.add)
            nc.vector.tensor_tensor(ttop[:], ttop[:], tbot[:], Alu.subtract)  # dy
            nc.vector.tensor_tensor(ttop[:], ttop[:], wyc[:], Alu.mult)
            nc.vector.tensor_tensor(ttop[:], ttop[:], tbot[:], Alu.add)       # res*255
            # stream->raster permute fused into the 1/255 scale copies.
            # rows 0-111: w64 pattern; 112-127: w32. Partition base must be
            # 0/32/64/96: do a w32-pattern op on [96,128) first, then fix
            # rows [96,112) with the w64 op (overwrite order matters).
            res = blend_pool.tile([128, 4096], f32, tag="res")
            AP64 = [[1024, 4], [1, 16], [16, 64]]
            AP32 = [[512, 8], [1, 16], [16, 32]]
            plan = [(96, 32, AP32, 1), (0, 32, AP64, 0), (32, 32, AP64, 1),
                    (64, 32, AP64, 0), (96, 16, AP64, 1)]
            for gi, (r0, nr, pat, eng) in enumerate(plan):
                sub = ttop[r0:r0 + nr, :]
                src_ap = _ap(sub, [sub.ap[0]] + pat)
                dsub = res[r0:r0 + nr, :]
                dst_ap = _ap(dsub, [dsub.ap[0]] + [[pat[0][0], pat[0][1]],
                                                   [pat[2][0] * pat[2][1] // 16 if False else pat[2][1], 16],
                                                   [1, pat[2][1]]])
                # dst pattern: (yq, r, x) -> raster: steps (16*w, w, 1)
                w = pat[2][1]
                nyq = pat[0][1]
                dst_ap = _ap(dsub, [dsub.ap[0], [16 * w, nyq], [w, 16], [1, w]])
                if eng:
                    nc.scalar.activation(dst_ap, src_ap, mybir.ActivationFunctionType.Copy,
                                         bias=0.0, scale=1.0 / 255.0)
                else:
                    nc.vector.tensor_scalar(dst_ap, src_ap, 1.0 / 255.0, None, Alu.mult)

            # ---- merged out DMAs (raster rows in res) ----
            for half in range(2):
                p = 2 * pp + half
                w64b = 56 * half
                w32b = 112 + 8 * half
                # fulls: 7 by-rows of 7 slots
                for byr in range(7):
                    s0 = w64b + 7 * byr
                    sub = res[s0:s0 + 7, :]
                    srcap = _ap(sub, [sub.ap[0], [64, 64], [1, 64]])
                    dst = _ap(out_ext[:], [[64, 7], [W, 64], [1, 64]],
                              off=p * PIX + (32 + 64 * byr) * W + 32)
                    nc.scalar.dma_start(dst, srcap)
                # row-halves: slots w64b+49..55, members (0,bx)@0, (8,bx)@2048
                for (o, y0) in ((0, 0), (2048, 480)):
                    sub = res[w64b + 49:w64b + 56, :]
                    srcap = _ap(sub, [sub.ap[0], [64, 32], [1, 64]], off=o)
                    dst = _ap(out_ext[:], [[64, 7], [W, 32], [1, 64]],
                              off=p * PIX + y0 * W + 32)
                    nc.scalar.dma_start(dst, srcap)
                # col-halves: slots w32b..w32b+6, members (by,0)@0, (by,8)@2048
                for (o, x0) in ((0, 0), (2048, 480)):
                    sub = res[w32b:w32b + 7, :]
                    srcap = _ap(sub, [sub.ap[0], [32, 64], [1, 32]], off=o)
                    dst = _ap(out_ext[:], [[64 * W, 7], [W, 64], [1, 32]],
                              off=p * PIX + 32 * W + x0)
                    nc.scalar.dma_start(dst, srcap)
                # quarters: slot w32b+7, 4 members
                for qi, (y0, x0) in enumerate(((0, 0), (0, 480), (480, 0), (480, 480))):
                    sub = res[w32b + 7:w32b + 8, :]
                    srcap = _ap(sub, [sub.ap[0], [32, 32], [1, 32]], off=1024 * qi)
                    dst = _ap(out_ext[:], [[1, 1], [W, 32], [1, 32]],
                              off=p * PIX + y0 * W + x0)
                    nc.scalar.dma_start(dst, srcap)

    fixup_sync(nc)
    return nc


# ======================= public entry point =======================
_CACHE = {}

def _get_nc():
    if 'nc' not in _CACHE:
        import ml_dtypes
        _CACHE['nc'] = build(debug=False)
        wx, wy = make_weight_consts()
        _CACHE['wx'] = wx.astype(ml_dtypes.bfloat16)
        _CACHE['wy'] = wy.astype(ml_dtypes.bfloat16)
    return _CACHE['nc'], _CACHE['wx'], _CACHE['wy']

def kernel(img: np.ndarray) -> np.ndarray:
    """CLAHE on trn2: img [32, 3, 512, 512] f32 -> same shape."""
    from concourse.bass_utils import run_bass_kernel_spmd
    img = np.asarray(img, dtype=np.float32)
    B, C, Hh, Ww = img.shape
    assert (B, C, Hh, Ww) == (32, 3, 512, 512)
    nc, wx, wy = _get_nc()
    shards = img.reshape(8, 4 * 3, 512, 512)
    in_maps = [dict(img=shards[i], wxc=wx, wyc=wy) for i in range(8)]
    res = run_bass_kernel_spmd(nc, in_maps, list(range(8)))
    out = np.stack([res.results[i]["out"] for i in range(8)], 0)
    return out.reshape(32, 3, 512, 512)

